# revision 22
# baseline (speedup 1.0000x reference)
"""CenterLoss Trainium2 kernel (Bass/Tile, 8 NeuronCores).

Full computation:
    count[c]  = histogram of ys over 1000 classes
    dist[i]   = || xs[i] - center[ys[i]] ||_2
    loss      = sum_i dist[i] / count[ys[i]]
              = sum_c ( sum_{i: ys[i]=c} dist[i] ) / count[c]

Sharding: data-parallel over the batch dim N=65536 across 8 NeuronCores
(8192 samples/core); the [1000, 512] center table is replicated. Each core
produces per-class partial distance sums S_c and partial counts n_c for
its shard; the host all-reduces (sums) the per-core partials and applies
the final per-class normalization sum_c S_c / n_c (~2k flops).

Per-core device pipeline
------------------------
A pipelined prologue casts the replicated center table to fp8-e4m3 into an
internal DRAM tensor (quarters the row-gather HBM traffic; measured
quantization error on the final loss vs the f32 reference is ~2e-4
relative). The shard is processed in 16 groups of 4 tiles (128 samples
per tile):
  1. xs group [128, 4, 512] f32 loaded via 4 rearranged-view DMAs.
  2. center[ys] row gather: one indirect DMA per tile ([P, 1] row offsets
     - the only offset shape the HW DGE expands correctly; multi-offset
     APs were verified broken on HW). The first 3 groups gather f32
     straight from the input table so the gather stream starts before the
     fp8 cast lands; later groups gather fp8 (1/4 the bytes).
  3. Per tile: DVE subtract (f32 - fp8), ACT Square with accum_out
     (per-sample sum of squares along features), then one ACT sqrt per
     group -> per-sample distances.
  4. Per-class binning WITHOUT any scatter, via a factorized one-hot
     matmul:  onehot(y)[1024] = onehot_lo(y%128)[128] (x) onehot_hi(y//128)[8]
     lhsT = onehot_lo [128samples, 128], rhs = [hi*dist, hi] [128samples, 16]
     -> PE accumulates PSUM [128, 16] across all 64 tiles (f32 matmul of
     0/1 x f32 values, exact). The one-hot planes are built group-wide on
     DVE from iota tables with per-partition is_equal compares.
Output per core: [128, 16] f32; cols 0:8 are distance-sum bins, 8:16 are
count bins; class c lives at [c % 128, 8 * part + c // 128].

Cost-model timeline (TimelineSim, per core): ~83 us, bounded by the 64
indirect-gather instructions on the GpSimd queue (~1.04 us fixed SWDGE
cost each) overlapped with ~68 us of DMA traffic (16 MiB xs + ~5 MiB
gathers + prologue) and ~55 us each of ACT/DVE elementwise work.
"""

import sys

import numpy as np

if "/opt/trn_rl_repo" not in sys.path:
    sys.path.insert(0, "/opt/trn_rl_repo")

N = 65536
F = 512
CLS = 1000
N_CORES = 8
SHARD = N // N_CORES  # 8192
P = 128
TILES = SHARD // P  # 64
GT = 4  # tiles per group
GROUPS = TILES // GT  # 16
XS_CHUNKS = 4  # DMA instructions per xs group (finer DMA-queue interleave)
N_F32_GROUPS = 3  # leading groups that gather f32 while the fp8 cast lands

_compiled = None


def _build():
    from concourse import bacc, bass, mybir, tile

    f32 = mybir.dt.float32
    i32 = mybir.dt.int32
    fp8 = mybir.dt.float8e4

    nc = bacc.Bacc(
        "TRN2",
        target_bir_lowering=False,
        debug=False,
        enable_asserts=False,
        num_devices=N_CORES,
        dynamic_dma_scratch_size=49152,
    )

    xs_d = nc.dram_tensor("xs", [SHARD, F], f32, kind="ExternalInput")
    yidx_d = nc.dram_tensor("yidx", [P, TILES], i32, kind="ExternalInput")
    ylo_d = nc.dram_tensor("ylo", [P, TILES], f32, kind="ExternalInput")
    yhi_d = nc.dram_tensor("yhi", [P, TILES], f32, kind="ExternalInput")
    center_d = nc.dram_tensor("center", [CLS, F], f32, kind="ExternalInput")
    c8_d = nc.dram_tensor("c8", [CLS, F], fp8, kind="Internal")
    out_d = nc.dram_tensor("out", [P, 16], f32, kind="ExternalOutput")

    sq_act = mybir.ActivationFunctionType.Square
    sqrt_act = mybir.ActivationFunctionType.Sqrt
    eq = mybir.AluOpType.is_equal

    with tile.TileContext(nc) as tc:
        with (
            tc.tile_pool(name="const", bufs=1) as const_pool,
            tc.tile_pool(name="xs", bufs=4) as xs_pool,
            tc.tile_pool(name="ce", bufs=10) as ce_pool,
            tc.tile_pool(name="diff", bufs=5) as diff_pool,
            tc.tile_pool(name="sq", bufs=1) as sq_pool,
            tc.tile_pool(name="lo", bufs=2) as lo_pool,
            tc.tile_pool(name="rhs", bufs=2) as rhs_pool,
            tc.tile_pool(name="small", bufs=4) as small_pool,
            tc.tile_pool(name="cast", bufs=1) as cast_pool,
            tc.tile_pool(name="psum", bufs=1, space="PSUM") as psum_pool,
        ):
            # Index plane for the gathers first — gathers need it at t~3us.
            yidx_sb = const_pool.tile([P, TILES], i32)
            nc.sync.dma_start(yidx_sb[:], yidx_d[:])

            # Prologue: cast center f32 -> fp8-e4m3 in DRAM, pipelined in
            # three chunks (2x [P,4|3,F] + ragged tail) so the fp8 table is
            # ready before the first fp8 gather. The cast loads are emitted
            # before the xs loads so they win the DMA queue.
            cast_parts = []
            for r0, r1, qn in ((0, 512, 4), (512, 896, 3), (896, CLS, 1)):
                rows = r1 - r0
                if qn > 1:
                    cf_t = cast_pool.tile([P, qn, F], f32, tag=f"cf{r0}")
                    nc.sync.dma_start(
                        cf_t[:, 0 : rows // P, :],
                        center_d[r0:r1, :].rearrange("(q p) d -> p q d", p=P),
                    )
                    cast_parts.append((r0, r1, rows // P, cf_t))
                else:
                    cf_t = cast_pool.tile([P, F], f32, tag=f"cf{r0}")
                    nc.sync.dma_start(cf_t[0:rows, :], center_d[r0:r1, :])
                    cast_parts.append((r0, r1, 0, cf_t))

            iota_lo = const_pool.tile([P, GT, P], f32)
            nc.gpsimd.iota(
                iota_lo[:],
                pattern=[[0, GT], [1, P]],
                base=0,
                channel_multiplier=0,
                allow_small_or_imprecise_dtypes=True,
            )
            iota_hi = const_pool.tile([P, GT, 8], f32)
            nc.gpsimd.iota(
                iota_hi[:],
                pattern=[[0, GT], [1, 8]],
                base=0,
                channel_multiplier=0,
                allow_small_or_imprecise_dtypes=True,
            )
            ylo_sb = const_pool.tile([P, TILES], f32)
            nc.sync.dma_start(ylo_sb[:], ylo_d[:])
            yhi_sb = const_pool.tile([P, TILES], f32)
            nc.sync.dma_start(yhi_sb[:], yhi_d[:])

            for r0, r1, qn, cf_t in cast_parts:
                rows = r1 - r0
                if qn:
                    c8_t = cast_pool.tile([P, qn, F], fp8, tag=f"c8{r0}")
                    nc.vector.tensor_copy(c8_t[:, 0:qn, :], cf_t[:, 0:qn, :])
                    nc.sync.dma_start(
                        c8_d[r0:r1, :].rearrange("(q p) d -> p q d", p=P),
                        c8_t[:, 0:qn, :],
                    )
                else:
                    c8_t = cast_pool.tile([P, F], fp8, tag=f"c8{r0}")
                    nc.vector.tensor_copy(c8_t[0:rows, :], cf_t[0:rows, :])
                    nc.sync.dma_start(c8_d[r0:r1, :], c8_t[0:rows, :])

            acc = psum_pool.tile([P, 16], f32)

            for g in range(GROUPS):
                gs = slice(g * GT, (g + 1) * GT)

                xs_b = xs_pool.tile([P, GT, F], f32)
                for h in range(XS_CHUNKS):
                    w = GT // XS_CHUNKS
                    r0 = (g * GT + h * w) * P
                    r1 = (g * GT + (h + 1) * w) * P
                    nc.sync.dma_start(
                        xs_b[:, h * w : (h + 1) * w, :],
                        xs_d[r0:r1, :].rearrange("(q p) d -> p q d", p=P),
                    )

                # One indirect row-gather per 128-sample tile, writing
                # disjoint slices of the group tile.
                if g < N_F32_GROUPS:
                    ce_b = ce_pool.tile(
                        [P, GT, F], f32, tag="ce32", bufs=N_F32_GROUPS
                    )
                    src_tab = center_d
                else:
                    ce_b = ce_pool.tile([P, GT, F], fp8)
                    src_tab = c8_d
                for q in range(GT):
                    t = g * GT + q
                    nc.gpsimd.indirect_dma_start(
                        out=ce_b[:, q, :],
                        out_offset=None,
                        in_=src_tab[:],
                        in_offset=bass.IndirectOffsetOnAxis(
                            ap=yidx_sb[:, t : t + 1], axis=0
                        ),
                    )

                dsq = small_pool.tile([P, GT], f32)
                for q in range(GT):
                    diff_t = diff_pool.tile([P, F], f32)
                    nc.vector.tensor_tensor(
                        out=diff_t[:], in0=xs_b[:, q, :], in1=ce_b[:, q, :],
                        op=mybir.AluOpType.subtract,
                    )
                    sq_t = sq_pool.tile([P, F], f32)
                    nc.scalar.activation(
                        out=sq_t[:], in_=diff_t[:],
                        func=sq_act, accum_out=dsq[:, q : q + 1],
                    )

                lo_b = lo_pool.tile([P, GT, P], f32)
                nc.vector.tensor_tensor(
                    out=lo_b[:],
                    in0=iota_lo[:],
                    in1=ylo_sb[:, gs].unsqueeze(2).broadcast_to([P, GT, P]),
                    op=eq,
                )
                rhs_b = rhs_pool.tile([P, GT, 16], f32)
                nc.vector.tensor_tensor(
                    out=rhs_b[:, :, 8:16],
                    in0=iota_hi[:],
                    in1=yhi_sb[:, gs].unsqueeze(2).broadcast_to([P, GT, 8]),
                    op=eq,
                )

                # Last group: per-tile sqrt/scale/matmul so the final matmul
                # chain doesn't wait for the whole group's distances.
                qchunk = 1 if g == GROUPS - 1 else GT
                dist = small_pool.tile([P, GT], f32)
                for q0 in range(0, GT, qchunk):
                    q1 = q0 + qchunk
                    nc.scalar.activation(
                        out=dist[:, q0:q1], in_=dsq[:, q0:q1], func=sqrt_act
                    )
                    nc.vector.tensor_tensor(
                        out=rhs_b[:, q0:q1, 0:8],
                        in0=rhs_b[:, q0:q1, 8:16],
                        in1=dist[:, q0:q1]
                        .unsqueeze(2)
                        .broadcast_to([P, q1 - q0, 8]),
                        op=mybir.AluOpType.mult,
                    )
                    for q in range(q0, q1):
                        t = g * GT + q
                        nc.tensor.matmul(
                            out=acc[:], lhsT=lo_b[:, q, :], rhs=rhs_b[:, q, :],
                            start=(t == 0), stop=(t == TILES - 1),
                        )

            out_sb = const_pool.tile([P, 16], f32)
            nc.vector.tensor_copy(out_sb[:], acc[:])
            nc.sync.dma_start(out_d[:], out_sb[:])

    nc.compile()
    return nc


def _get_compiled():
    global _compiled
    if _compiled is None:
        _compiled = _build()
    return _compiled


def _make_in_maps(xs, ys, center):
    in_maps = []
    for c in range(N_CORES):
        ys_c = ys[c * SHARD : (c + 1) * SHARD]
        # yidx[p, t] = label of sample t*128+p of this core's shard.
        yidx = np.ascontiguousarray(ys_c.reshape(TILES, P).T)
        in_maps.append(
            {
                "xs": xs[c * SHARD : (c + 1) * SHARD],
                "yidx": yidx,
                "ylo": np.ascontiguousarray((yidx % P).astype(np.float32)),
                "yhi": np.ascontiguousarray((yidx // P).astype(np.float32)),
                "center": center,
            }
        )
    return in_maps


def kernel(xs, ys, center):
    from concourse.bass_utils import run_bass_kernel_spmd

    xs = np.ascontiguousarray(np.asarray(xs), dtype=np.float32)
    ys = np.asarray(ys).astype(np.int32)
    center = np.ascontiguousarray(np.asarray(center), dtype=np.float32)

    nc = _get_compiled()
    in_maps = _make_in_maps(xs, ys, center)
    res = run_bass_kernel_spmd(nc, in_maps, core_ids=list(range(N_CORES)))

    # All-reduce the per-core per-class partials, then the final
    # per-class normalization: loss = sum_c S_c / n_c.
    total = np.zeros((P, 16), dtype=np.float64)
    for r in res.results:
        total += r["out"].astype(np.float64)
    s_bins = total[:, 0:8].T.reshape(-1)  # class c at index c
    n_bins = total[:, 8:16].T.reshape(-1)
    mask = n_bins > 0
    loss = (s_bins[mask] / n_bins[mask]).sum()
    return np.float32(loss)
